# revision 52
# baseline (speedup 1.0000x reference)
"""3x3 median filter (reflect padding) on Trainium2, 8-core data parallel.

Layout (per core, 4 images):
  partition p = b*32 + g
    b in 0..3  : image index within the core's batch shard
    g in 0..31 : group of 7 consecutive output rows
  linear(p) = p*7*W*C addresses (b,g) jointly (the strides nest
  perfectly), so one 3-dim access pattern spans all 128 partitions.

All device compute is fp16 (host converts): 2-byte packed operands put
the DVE in its 2x perf mode (measured 0.553 ns/elem sustained; fp32
and u8 both run 1x).  Multi-row (3D) access patterns cost ~95ns per
row segment on the DVE, so every big op is FLAT (single segment
spanning rows); the merge stage's +-3 shifts then bleed across row
boundaries, but only into columns >= 666 of each row, which no
consumer reads (the final med3 consumes cols 0..665 and the image's
first/last output columns are recomputed exactly by the edge block).

Loads: DMA pieces must span ~64 partitions to spread across the 16
DMA engines (narrow pieces serialize onto ONE engine, ~13x slower).
Wave A1 carries slab rows 1..3 (dram 0..2): in-bounds everywhere, no
over-read, no reflect involvement, so the first compute op waits on
nothing but A1 (~12us, mostly the fixed ~7us NEFF preamble plus the
A1 transfer).  A2 = slab rows 4..6, then the 1-row slab-row-0 piece
(dram -1; the image-boundary partitions get garbage there) plus a
4-partition reflect patch whose WAW-wait on the row-0 piece resolves
off the critical path, then wave B = slab rows 7..8 (over-read at the
bottom boundaries, fixed by the same kind of reflect patch).  Wave
sizing is tuned so each phase's compute covers the next wave's
arrival; the DVE stream then runs gap-free from first op to last.

Median of 9 = med3( max3(col_lows), med3(col_meds), min3(col_highs) )
with each vertical column triple sorted once (P/Q pair min/max then
lo/med/hi, 6N flat ops) and shared across the three horizontally
adjacent windows.  Horizontal neighbor access is a +-3 float shift
inside each row; the first/last output columns are recomputed exactly
by the narrow edge-column block (which slots for free into the merge
stage's pipeline shadows).  The final med3 chain + store go out in
four row groups (3/2/1/1 rows) so stores overlap the tail compute and
the last store is a single row.

Engine notes from this tuning session: the DVE is the ONLY engine
that can do elementwise min/max on TRN2 (TensorTensor is rejected on
Pool by the CoreV3 ISA; ACT's activation is unary with per-partition
scalar bias; DMA accumulate supports neither min nor max; PE+ACT
decompositions via min(a,b) = 0.5(a+b) - 0.5|a-b| cost more ACT/evac
time than they save).  Single-run HW times vary by up to ~20% from
clock throttling -- always compare best-of-several.

Measured floor decomposition (best run 64.1us): 12.1us head (7.2
fixed NEFF preamble + issue + A1 transfer at the DMA engines' cold
~120-150GB/s ramp rate; warm-up dummy DMAs only delay A1 and do not
help), 48.2us gap-free DVE stream (44.2us pure 2x-mode streaming +
~74ns/op instruction overhead; pair-sharing the vertical sort to
5.14N loses to the strided-row segment overhead it requires), 4.7us
tail (store issue + last 1-row transfer + 2.3us fixed teardown).
Final-group splits measured: 4 groups (3/2/1/1) beat 3 and 5 groups;
the two small tail groups store as single-queue pieces (one issue
instruction instead of the slower of two).  Loading the late waves as
u8 with gpsimd cast-DMA halves their bytes but the software DGE
cannot sustain bulk bandwidth (+11us) -- keep all loads on the two
HW-DGE queues.  The device throttles its clock ~17% under sustained
load and recovers after ~1-2min idle; compare best-of-several runs.
"""

import sys

if "/opt/trn_rl_repo" not in sys.path:
    sys.path.insert(0, "/opt/trn_rl_repo")

import numpy as np

import concourse.bass as bass  # noqa: F401
import concourse.tile as tile
from concourse import bacc, mybir
from concourse.ap import AP
from concourse.bass_utils import run_bass_kernel_spmd

F32 = mybir.dt.float32
F16 = mybir.dt.float16
MIN = mybir.AluOpType.min
MAX = mybir.AluOpType.max

B, H, W, C = 32, 224, 224, 3
NCORES = 8
BPC = B // NCORES      # 4 images per core
NG, GR = 32, 7         # row-groups per image, rows per group
WC = W * C             # 672 floats per image row
IMG = H * WC
PS = GR * WC           # 4704: per-partition linear stride
R = GR                 # 7 output rows per partition
N = R * WC             # 4704 output floats per partition
SRR = R + 2            # 9 slab rows

_CACHE = {}


def _build_kernel(tc, y, x):
    nc = tc.nc
    qa, qb = nc.sync, nc.scalar

    with tc.tile_pool(name="sb", bufs=1) as sb:
        S = sb.tile([128, SRR, WC], F16, tag="s", name="S")

        def rows(q, p0, p1, dram_row, s0, nr):
            q.dma_start(S[p0:p1, s0:s0 + nr, :],
                        AP(x.tensor, p0 * PS + dram_row * WC,
                           [[PS, p1 - p0], [1, nr * WC]]))

        # ---- loads ------------------------------------------------
        # wave A1: slab rows 1..3 (dram 0..2)
        rows(qa, 0, 64, 0, 1, 3)
        rows(qb, 64, 128, 0, 1, 3)
        # wave A2: slab rows 4..6 (dram 3..5)
        rows(qa, 0, 64, 3, 4, 3)
        rows(qb, 64, 128, 3, 4, 3)
        # slab row 0 (dram -1) for p >= 1; boundary partitions get
        # garbage here, fixed by the reflect top patch below
        rows(qa, 1, 64, -1, 0, 1)
        rows(qb, 64, 128, -1, 0, 1)
        # reflect top (slab row 0 at p = 0,32,64,96 <- image row 1);
        # WAW-wait on the row-0 pieces resolves ~17us, well before the
        # (0,1) pair ops need it (~26us)
        qb.dma_start(S[0:128:32, 0:1, :],
                     AP(x.tensor, WC, [[IMG, 4], [1, WC]]))
        # wave B: slab rows 7..8 (dram 6..7, over-read at the bottom
        # boundaries)
        rows(qa, 0, 64, 6, 7, 2)
        rows(qb, 64, 127, 6, 7, 2)
        qb.dma_start(S[127:128, 7:8, :],     # p127 slab row 7
                     AP(x.tensor, 127 * PS + 6 * WC, [[1, WC]]))
        # reflect bottom (slab row 8 at p = 31,63,95,127 <- image row
        # 222); WAW-wait on wave B resolves well before pairs m3
        qa.dma_start(S[31:128:32, 8:9, :],
                     AP(x.tensor, (H - 2) * WC, [[IMG, 4], [1, WC]]))

        Sf = S.rearrange("p r f -> p (r f)")

        # ---- stage 1: vertical column sort (flat ops) --------------
        # P/Q[k] = min/max(S[k], S[k+1]); LO/ME/HI[k] = sorted triple
        # (k, k+1, k+2), range-split to chase the arriving waves.
        P = sb.tile([128, N], F16, tag="p", name="P")
        Q = sb.tile([128, N], F16, tag="q", name="Q")
        LO = sb.tile([128, R, WC], F16, tag="lo", name="LO")
        ME = sb.tile([128, R, WC], F16, tag="me", name="ME")
        HI = sb.tile([128, R, WC], F16, tag="hi", name="HI")
        T1 = sb.tile([128, R, WC], F16, tag="t1", name="T1")
        LOf = LO.rearrange("p r f -> p (r f)")
        MEf = ME.rearrange("p r f -> p (r f)")
        HIf = HI.rearrange("p r f -> p (r f)")
        T1f = T1.rearrange("p r f -> p (r f)")

        def s1_pq(fa, fb):
            nc.vector.tensor_tensor(P[:, fa:fb], Sf[:, fa:fb],
                                    Sf[:, fa + WC:fb + WC], MIN)
            nc.vector.tensor_tensor(Q[:, fa:fb], Sf[:, fa:fb],
                                    Sf[:, fa + WC:fb + WC], MAX)

        def s1_cols(fa, fb):
            nc.vector.tensor_tensor(LOf[:, fa:fb], P[:, fa:fb],
                                    Sf[:, fa + 2 * WC:fb + 2 * WC], MIN)
            nc.vector.tensor_tensor(T1f[:, fa:fb], Q[:, fa:fb],
                                    Sf[:, fa + 2 * WC:fb + 2 * WC], MIN)
            nc.vector.tensor_tensor(HIf[:, fa:fb], Q[:, fa:fb],
                                    Sf[:, fa + 2 * WC:fb + 2 * WC], MAX)
            nc.vector.tensor_tensor(MEf[:, fa:fb], P[:, fa:fb],
                                    T1f[:, fa:fb], MAX)

        # A1 (slab rows 1..3): pairs (1,2), (2,3)
        s1_pq(WC, 3 * WC)
        # triple at output row 1 (slab 1,2,3) -- A1 only
        s1_cols(WC, 2 * WC)
        # A2 (slab rows 4..6): pairs (3,4),(4,5),(5,6)
        s1_pq(3 * WC, 6 * WC)
        # pair (0,1) -- needs slab row 0 (+ copies)
        s1_pq(0, WC)
        # triples at output rows 0, 2..4 (slab rows <= 6)
        s1_cols(0, WC)
        s1_cols(2 * WC, 5 * WC)
        # B (slab rows 7..8): pair (6,7)
        s1_pq(6 * WC, N)
        # triples at output rows 5..6
        s1_cols(5 * WC, N)

        M1 = sb.tile([128, R, WC], F16, tag="m1", name="M1")

        # ---- exact first/last output columns (reflect), per row
        # group AFTER that group's flat final (whose cross-row bleed
        # writes garbage into cols 0..2 of interior rows); col 0:
        # window cols (1,0,1) -> med3(max(lo0,lo1), med1,
        # min(hi0,hi1)); col 223: window cols (222,223,222).
        L4 = LO.rearrange("p r (a c) -> p r a c", a=W, c=C)
        H4 = HI.rearrange("p r (a c) -> p r a c", a=W, c=C)
        T4 = ME.rearrange("p r (a c) -> p r a c", a=W, c=C)
        M4 = M1.rearrange("p r (a c) -> p r a c", a=W, c=C)
        ae = sb.tile([128, R, 2, C], F16, tag="ae", name="ae")
        ce = sb.tile([128, R, 2, C], F16, tag="ce", name="ce")
        mem = sb.tile([128, R, 2, C], F16, tag="mm", name="mm")

        lo_o = L4[:, :, 0:W:W - 1, :]      # cols {0, 223}
        lo_i = L4[:, :, 1:W:W - 3, :]      # cols {1, 222}
        hi_o = H4[:, :, 0:W:W - 1, :]
        hi_i = H4[:, :, 1:W:W - 3, :]
        be = T4[:, :, 1:W:W - 3, :]        # med of inner col
        nc.vector.tensor_tensor(ae[:], lo_o, lo_i, MAX)
        nc.vector.tensor_tensor(ce[:], hi_o, hi_i, MIN)
        nc.vector.tensor_tensor(mem[:], ae[:], be, MIN)
        nc.vector.tensor_tensor(ae[:], ae[:], be, MAX)
        nc.vector.tensor_tensor(ce[:], ae[:], ce[:], MIN)
        nc.vector.tensor_tensor(M4[:, :, 0:W:W - 1, :], mem[:], ce[:], MAX)

        # ---- stage 2: horizontal merge, all FLAT single-segment ops
        # over the whole 7-row slab; the +-3/-6 shifts bleed across
        # row boundaries but only into per-row columns >= 666, which
        # nothing consumes.
        E = N - 3
        D = N - 6
        U = sb.tile([128, R, WC], F16, tag="u", name="U")
        V = sb.tile([128, R, WC], F16, tag="v", name="V")
        Sm = sb.tile([128, R, WC], F16, tag="sm", name="Sm")
        Tm = sb.tile([128, R, WC], F16, tag="tm", name="Tm")
        MT = sb.tile([128, R, WC], F16, tag="mt", name="MT")
        Uf = U.rearrange("p r f -> p (r f)")
        Vf = V.rearrange("p r f -> p (r f)")
        Smf = Sm.rearrange("p r f -> p (r f)")
        Tmf = Tm.rearrange("p r f -> p (r f)")
        MTf = MT.rearrange("p r f -> p (r f)")

        nc.vector.tensor_tensor(Uf[:, 0:E], LOf[:, 0:E], LOf[:, 3:N], MAX)
        nc.vector.tensor_tensor(Vf[:, 0:E], HIf[:, 0:E], HIf[:, 3:N], MIN)
        nc.vector.tensor_tensor(Smf[:, 0:E], MEf[:, 0:E], MEf[:, 3:N], MIN)
        nc.vector.tensor_tensor(Tmf[:, 0:E], MEf[:, 0:E], MEf[:, 3:N], MAX)
        nc.vector.tensor_tensor(Uf[:, 0:D], Uf[:, 0:D], LOf[:, 6:N], MAX)
        nc.vector.tensor_tensor(Vf[:, 0:D], Vf[:, 0:D], HIf[:, 6:N], MIN)
        nc.vector.tensor_tensor(Tmf[:, 0:D], Tmf[:, 0:D], MEf[:, 6:N], MIN)
        nc.vector.tensor_tensor(Smf[:, 0:D], Smf[:, 0:D], Tmf[:, 0:D], MAX)

        A = Uf   # max3 of lows
        Cc = Vf  # min3 of highs
        Bm = Smf  # med3 of meds

        # ---- final med3 chain + store in 3 row groups (3D output
        # APs: no cross-row garbage may touch the edge columns that
        # the edge block already wrote)
        def final(ra, rb, queues=((0, 64, qa), (64, 128, qb))):
            nc.vector.tensor_tensor(MT[:, ra:rb, 0:WC - 6],
                                    U[:, ra:rb, 0:WC - 6],
                                    Sm[:, ra:rb, 0:WC - 6], MIN)
            nc.vector.tensor_tensor(U[:, ra:rb, 0:WC - 6],
                                    U[:, ra:rb, 0:WC - 6],
                                    Sm[:, ra:rb, 0:WC - 6], MAX)
            nc.vector.tensor_tensor(V[:, ra:rb, 0:WC - 6],
                                    U[:, ra:rb, 0:WC - 6],
                                    V[:, ra:rb, 0:WC - 6], MIN)
            nc.vector.tensor_tensor(M1[:, ra:rb, 3:WC - 3],
                                    MT[:, ra:rb, 0:WC - 6],
                                    V[:, ra:rb, 0:WC - 6], MAX)
            for (p0, p1, q) in queues:
                dst = AP(y.tensor, p0 * PS + ra * WC,
                         [[PS, p1 - p0], [WC, rb - ra], [1, WC]])
                q.dma_start(dst, M1[p0:p1, ra:rb, :])

        final(0, 3)
        final(3, 5)
        # last two rows as ONE compute group (4 bigger ops finish
        # ~0.26us earlier than 2x4 small ops) with two parallel 1-row
        # single-queue stores that fire together off the last op
        final(5, 7, queues=())
        qb.dma_start(AP(y.tensor, 5 * WC, [[PS, 128], [1, WC]]),
                     M1[:, 5:6, :])
        qa.dma_start(AP(y.tensor, 6 * WC, [[PS, 128], [1, WC]]),
                     M1[:, 6:7, :])


def _build():
    if "nc" in _CACHE:
        return _CACHE["nc"]
    nc = bacc.Bacc("TRN2", target_bir_lowering=False, debug=False)
    x = nc.dram_tensor("x", [BPC, H, W, C], F16, kind="ExternalInput").ap()
    y = nc.dram_tensor("y", [BPC, H, W, C], F16, kind="ExternalOutput").ap()
    with tile.TileContext(nc) as tc:
        _build_kernel(tc, y, x)
    nc.compile()
    _CACHE["nc"] = nc
    return nc


def run(input_batch, **spmd_kwargs):
    nc = _build()
    xh = np.ascontiguousarray(input_batch).astype(np.float16)
    in_maps = [
        {"x": np.ascontiguousarray(xh[i * BPC:(i + 1) * BPC])}
        for i in range(NCORES)
    ]
    res = run_bass_kernel_spmd(nc, in_maps, list(range(NCORES)), **spmd_kwargs)
    out = np.concatenate([r["y"] for r in res.results],
                         axis=0).astype(np.float32)
    return out, res


def kernel(input_batch):
    out, _ = run(np.asarray(input_batch))
    return out


# revision 53
# speedup vs baseline: 1.0097x; 1.0097x over previous
"""3x3 median filter (reflect padding) on Trainium2, 8-core data parallel.

Layout (per core, 4 images):
  partition p = b*32 + g
    b in 0..3  : image index within the core's batch shard
    g in 0..31 : group of 7 consecutive output rows
  linear(p) = p*7*W*C addresses (b,g) jointly (the strides nest
  perfectly), so one 3-dim access pattern spans all 128 partitions.

All device compute is fp16 (host converts): 2-byte packed operands put
the DVE in its 2x perf mode (measured 0.553 ns/elem sustained; fp32
and u8 both run 1x).  Multi-row (3D) access patterns cost ~95ns per
row segment on the DVE, so every big op is FLAT (single segment
spanning rows); the merge stage's +-3 shifts then bleed across row
boundaries, but only into columns >= 666 of each row, which no
consumer reads (the final med3 consumes cols 0..665 and the image's
first/last output columns are recomputed exactly by the edge block).

Loads: DMA pieces must span ~64 partitions to spread across the 16
DMA engines (narrow pieces serialize onto ONE engine, ~13x slower).
Wave A1 carries slab rows 1..3 (dram 0..2): in-bounds everywhere, no
over-read, no reflect involvement, so the first compute op waits on
nothing but A1 (~12us, mostly the fixed ~7us NEFF preamble plus the
A1 transfer).  A2 = slab rows 4..6, then the 1-row slab-row-0 piece
(dram -1; the image-boundary partitions get garbage there) plus a
4-partition reflect patch whose WAW-wait on the row-0 piece resolves
off the critical path, then wave B = slab rows 7..8 (over-read at the
bottom boundaries, fixed by the same kind of reflect patch).  Wave
sizing is tuned so each phase's compute covers the next wave's
arrival; the DVE stream then runs gap-free from first op to last.

Median of 9 = med3( max3(col_lows), med3(col_meds), min3(col_highs) )
with each vertical column triple sorted once (P/Q pair min/max then
lo/med/hi, 6N flat ops) and shared across the three horizontally
adjacent windows.  Horizontal neighbor access is a +-3 float shift
inside each row; the first/last output columns are recomputed exactly
by the narrow edge-column block (which slots for free into the merge
stage's pipeline shadows).  The final med3 chain + store go out in
four row groups (3/2/1/1 rows) so stores overlap the tail compute and
the last store is a single row.

Engine notes from this tuning session: the DVE is the ONLY engine
that can do elementwise min/max on TRN2 (TensorTensor is rejected on
Pool by the CoreV3 ISA; ACT's activation is unary with per-partition
scalar bias; DMA accumulate supports neither min nor max; PE+ACT
decompositions via min(a,b) = 0.5(a+b) - 0.5|a-b| cost more ACT/evac
time than they save).  Single-run HW times vary by up to ~20% from
clock throttling -- always compare best-of-several.

Measured floor decomposition (best run 64.1us): 12.1us head (7.2
fixed NEFF preamble + issue + A1 transfer at the DMA engines' cold
~120-150GB/s ramp rate; warm-up dummy DMAs only delay A1 and do not
help), 48.2us gap-free DVE stream (44.2us pure 2x-mode streaming +
~74ns/op instruction overhead; pair-sharing the vertical sort to
5.14N loses to the strided-row segment overhead it requires), 4.7us
tail (store issue + last 1-row transfer + 2.3us fixed teardown).
Final-group splits measured: 4 groups (3/2/1/1) beat 3 and 5 groups;
the two small tail groups store as single-queue pieces (one issue
instruction instead of the slower of two).  Loading the late waves as
u8 with gpsimd cast-DMA halves their bytes but the software DGE
cannot sustain bulk bandwidth (+11us) -- keep all loads on the two
HW-DGE queues.  The device throttles its clock ~17% under sustained
load and recovers after ~1-2min idle; compare best-of-several runs.
"""

import sys

if "/opt/trn_rl_repo" not in sys.path:
    sys.path.insert(0, "/opt/trn_rl_repo")

import numpy as np

import concourse.bass as bass  # noqa: F401
import concourse.tile as tile
from concourse import bacc, mybir
from concourse.ap import AP
from concourse.bass_utils import run_bass_kernel_spmd

F32 = mybir.dt.float32
F16 = mybir.dt.float16
MIN = mybir.AluOpType.min
MAX = mybir.AluOpType.max

B, H, W, C = 32, 224, 224, 3
NCORES = 8
BPC = B // NCORES      # 4 images per core
NG, GR = 32, 7         # row-groups per image, rows per group
WC = W * C             # 672 floats per image row
IMG = H * WC
PS = GR * WC           # 4704: per-partition linear stride
R = GR                 # 7 output rows per partition
N = R * WC             # 4704 output floats per partition
SRR = R + 2            # 9 slab rows

_CACHE = {}


def _build_kernel(tc, y, x):
    nc = tc.nc
    qa, qb = nc.sync, nc.scalar

    with tc.tile_pool(name="sb", bufs=1) as sb:
        S = sb.tile([128, SRR, WC], F16, tag="s", name="S")

        def rows(q, p0, p1, dram_row, s0, nr):
            q.dma_start(S[p0:p1, s0:s0 + nr, :],
                        AP(x.tensor, p0 * PS + dram_row * WC,
                           [[PS, p1 - p0], [1, nr * WC]]))

        # ---- loads ------------------------------------------------
        # wave A1: slab rows 1..3 (dram 0..2)
        rows(qa, 0, 64, 0, 1, 3)
        rows(qb, 64, 128, 0, 1, 3)
        # wave A2: slab rows 4..6 (dram 3..5)
        rows(qa, 0, 64, 3, 4, 3)
        rows(qb, 64, 128, 3, 4, 3)
        # slab row 0 (dram -1) for p >= 1; boundary partitions get
        # garbage here, fixed by the reflect top patch below
        rows(qa, 1, 64, -1, 0, 1)
        rows(qb, 64, 128, -1, 0, 1)
        # reflect top (slab row 0 at p = 0,32,64,96 <- image row 1);
        # WAW-wait on the row-0 pieces resolves ~17us, well before the
        # (0,1) pair ops need it (~26us)
        qb.dma_start(S[0:128:32, 0:1, :],
                     AP(x.tensor, WC, [[IMG, 4], [1, WC]]))
        # wave B: slab rows 7..8 (dram 6..7, over-read at the bottom
        # boundaries)
        rows(qa, 0, 64, 6, 7, 2)
        rows(qb, 64, 127, 6, 7, 2)
        qb.dma_start(S[127:128, 7:8, :],     # p127 slab row 7
                     AP(x.tensor, 127 * PS + 6 * WC, [[1, WC]]))
        # reflect bottom (slab row 8 at p = 31,63,95,127 <- image row
        # 222); WAW-wait on wave B resolves well before pairs m3
        qa.dma_start(S[31:128:32, 8:9, :],
                     AP(x.tensor, (H - 2) * WC, [[IMG, 4], [1, WC]]))

        Sf = S.rearrange("p r f -> p (r f)")

        # ---- stage 1: vertical column sort (flat ops) --------------
        # P/Q[k] = min/max(S[k], S[k+1]); LO/ME/HI[k] = sorted triple
        # (k, k+1, k+2), range-split to chase the arriving waves.
        P = sb.tile([128, N], F16, tag="p", name="P")
        Q = sb.tile([128, N], F16, tag="q", name="Q")
        LO = sb.tile([128, R, WC], F16, tag="lo", name="LO")
        ME = sb.tile([128, R, WC], F16, tag="me", name="ME")
        HI = sb.tile([128, R, WC], F16, tag="hi", name="HI")
        T1 = sb.tile([128, R, WC], F16, tag="t1", name="T1")
        LOf = LO.rearrange("p r f -> p (r f)")
        MEf = ME.rearrange("p r f -> p (r f)")
        HIf = HI.rearrange("p r f -> p (r f)")
        T1f = T1.rearrange("p r f -> p (r f)")

        def s1_pq(fa, fb):
            nc.vector.tensor_tensor(P[:, fa:fb], Sf[:, fa:fb],
                                    Sf[:, fa + WC:fb + WC], MIN)
            nc.vector.tensor_tensor(Q[:, fa:fb], Sf[:, fa:fb],
                                    Sf[:, fa + WC:fb + WC], MAX)

        def s1_cols(fa, fb):
            nc.vector.tensor_tensor(LOf[:, fa:fb], P[:, fa:fb],
                                    Sf[:, fa + 2 * WC:fb + 2 * WC], MIN)
            nc.vector.tensor_tensor(T1f[:, fa:fb], Q[:, fa:fb],
                                    Sf[:, fa + 2 * WC:fb + 2 * WC], MIN)
            nc.vector.tensor_tensor(HIf[:, fa:fb], Q[:, fa:fb],
                                    Sf[:, fa + 2 * WC:fb + 2 * WC], MAX)
            nc.vector.tensor_tensor(MEf[:, fa:fb], P[:, fa:fb],
                                    T1f[:, fa:fb], MAX)

        # A1 (slab rows 1..3): pairs (1,2), (2,3)
        s1_pq(WC, 3 * WC)
        # triple at output row 1 (slab 1,2,3) -- A1 only
        s1_cols(WC, 2 * WC)
        # A2 (slab rows 4..6): pairs (3,4),(4,5),(5,6)
        s1_pq(3 * WC, 6 * WC)
        # pair (0,1) -- needs slab row 0 (+ copies)
        s1_pq(0, WC)
        # triples at output rows 0, 2..4 (slab rows <= 6)
        s1_cols(0, WC)
        s1_cols(2 * WC, 5 * WC)
        # B (slab rows 7..8): pair (6,7)
        s1_pq(6 * WC, N)
        # triples at output rows 5..6
        s1_cols(5 * WC, N)

        M1 = sb.tile([128, R, WC], F16, tag="m1", name="M1")

        # ---- exact first/last output columns (reflect), per row
        # group AFTER that group's flat final (whose cross-row bleed
        # writes garbage into cols 0..2 of interior rows); col 0:
        # window cols (1,0,1) -> med3(max(lo0,lo1), med1,
        # min(hi0,hi1)); col 223: window cols (222,223,222).
        L4 = LO.rearrange("p r (a c) -> p r a c", a=W, c=C)
        H4 = HI.rearrange("p r (a c) -> p r a c", a=W, c=C)
        T4 = ME.rearrange("p r (a c) -> p r a c", a=W, c=C)
        M4 = M1.rearrange("p r (a c) -> p r a c", a=W, c=C)
        ae = sb.tile([128, R, 2, C], F16, tag="ae", name="ae")
        ce = sb.tile([128, R, 2, C], F16, tag="ce", name="ce")
        mem = sb.tile([128, R, 2, C], F16, tag="mm", name="mm")

        lo_o = L4[:, :, 0:W:W - 1, :]      # cols {0, 223}
        lo_i = L4[:, :, 1:W:W - 3, :]      # cols {1, 222}
        hi_o = H4[:, :, 0:W:W - 1, :]
        hi_i = H4[:, :, 1:W:W - 3, :]
        be = T4[:, :, 1:W:W - 3, :]        # med of inner col
        nc.vector.tensor_tensor(ae[:], lo_o, lo_i, MAX)
        nc.vector.tensor_tensor(ce[:], hi_o, hi_i, MIN)
        nc.vector.tensor_tensor(mem[:], ae[:], be, MIN)
        nc.vector.tensor_tensor(ae[:], ae[:], be, MAX)
        nc.vector.tensor_tensor(ce[:], ae[:], ce[:], MIN)
        nc.vector.tensor_tensor(M4[:, :, 0:W:W - 1, :], mem[:], ce[:], MAX)

        # ---- stage 2: horizontal merge, all FLAT single-segment ops
        # over the whole 7-row slab; the +-3/-6 shifts bleed across
        # row boundaries but only into per-row columns >= 666, which
        # nothing consumes.
        E = N - 3
        D = N - 6
        U = sb.tile([128, R, WC], F16, tag="u", name="U")
        V = sb.tile([128, R, WC], F16, tag="v", name="V")
        Sm = sb.tile([128, R, WC], F16, tag="sm", name="Sm")
        Tm = sb.tile([128, R, WC], F16, tag="tm", name="Tm")
        MT = sb.tile([128, R, WC], F16, tag="mt", name="MT")
        Uf = U.rearrange("p r f -> p (r f)")
        Vf = V.rearrange("p r f -> p (r f)")
        Smf = Sm.rearrange("p r f -> p (r f)")
        Tmf = Tm.rearrange("p r f -> p (r f)")
        MTf = MT.rearrange("p r f -> p (r f)")

        nc.vector.tensor_tensor(Uf[:, 0:E], LOf[:, 0:E], LOf[:, 3:N], MAX)
        nc.vector.tensor_tensor(Vf[:, 0:E], HIf[:, 0:E], HIf[:, 3:N], MIN)
        nc.vector.tensor_tensor(Smf[:, 0:E], MEf[:, 0:E], MEf[:, 3:N], MIN)
        nc.vector.tensor_tensor(Tmf[:, 0:E], MEf[:, 0:E], MEf[:, 3:N], MAX)
        nc.vector.tensor_tensor(Uf[:, 0:D], Uf[:, 0:D], LOf[:, 6:N], MAX)
        nc.vector.tensor_tensor(Vf[:, 0:D], Vf[:, 0:D], HIf[:, 6:N], MIN)
        nc.vector.tensor_tensor(Tmf[:, 0:D], Tmf[:, 0:D], MEf[:, 6:N], MIN)
        nc.vector.tensor_tensor(Smf[:, 0:D], Smf[:, 0:D], Tmf[:, 0:D], MAX)

        A = Uf   # max3 of lows
        Cc = Vf  # min3 of highs
        Bm = Smf  # med3 of meds

        # ---- final med3 chain + store in 3 row groups (3D output
        # APs: no cross-row garbage may touch the edge columns that
        # the edge block already wrote)
        def final(ra, rb, queues=((0, 64, qa), (64, 128, qb))):
            nc.vector.tensor_tensor(MT[:, ra:rb, 0:WC - 6],
                                    U[:, ra:rb, 0:WC - 6],
                                    Sm[:, ra:rb, 0:WC - 6], MIN)
            nc.vector.tensor_tensor(U[:, ra:rb, 0:WC - 6],
                                    U[:, ra:rb, 0:WC - 6],
                                    Sm[:, ra:rb, 0:WC - 6], MAX)
            nc.vector.tensor_tensor(V[:, ra:rb, 0:WC - 6],
                                    U[:, ra:rb, 0:WC - 6],
                                    V[:, ra:rb, 0:WC - 6], MIN)
            nc.vector.tensor_tensor(M1[:, ra:rb, 3:WC - 3],
                                    MT[:, ra:rb, 0:WC - 6],
                                    V[:, ra:rb, 0:WC - 6], MAX)
            for (p0, p1, q) in queues:
                dst = AP(y.tensor, p0 * PS + ra * WC,
                         [[PS, p1 - p0], [WC, rb - ra], [1, WC]])
                q.dma_start(dst, M1[p0:p1, ra:rb, :])

        final(0, 3)
        final(3, 5)
        # single-queue stores for the small tail groups: waiting on
        # one issue instruction instead of the slower of two.
        # Measured-worse tail alternatives: 3 and 5 row-groups,
        # column-split last row, merged (5,7) compute with parallel
        # 1-row stores -- the 3/2/1/1 cascade below beats them all.
        final(5, 6, queues=((0, 128, qb),))
        final(6, 7, queues=((0, 128, qa),))


def _build():
    if "nc" in _CACHE:
        return _CACHE["nc"]
    nc = bacc.Bacc("TRN2", target_bir_lowering=False, debug=False)
    x = nc.dram_tensor("x", [BPC, H, W, C], F16, kind="ExternalInput").ap()
    y = nc.dram_tensor("y", [BPC, H, W, C], F16, kind="ExternalOutput").ap()
    with tile.TileContext(nc) as tc:
        _build_kernel(tc, y, x)
    nc.compile()
    _CACHE["nc"] = nc
    return nc


def run(input_batch, **spmd_kwargs):
    nc = _build()
    xh = np.ascontiguousarray(input_batch).astype(np.float16)
    in_maps = [
        {"x": np.ascontiguousarray(xh[i * BPC:(i + 1) * BPC])}
        for i in range(NCORES)
    ]
    res = run_bass_kernel_spmd(nc, in_maps, list(range(NCORES)), **spmd_kwargs)
    out = np.concatenate([r["y"] for r in res.results],
                         axis=0).astype(np.float32)
    return out, res


def kernel(input_batch):
    out, _ = run(np.asarray(input_batch))
    return out
